# revision 1
# baseline (speedup 1.0000x reference)
"""CAMoE-GNN Trainium2 kernel (8 NeuronCores, SPMD).

Math (reference, per layer):
    gate = softmax((top @ Wg.T)/TEMP)            [N,3]
    he   = h @ W[e]
    agg  = segsum(he[src]*dinv[src]*dinv[dst] -> dst)   (incl. self loops)
    out  = sum_e gate_e * relu(agg_e + b[e])

Key algebra used here:
    aggregation commutes with W[e]:  agg_e = (A @ h) @ W[e]  with
    A = D^-1/2 (M + I) D^-1/2.  So the sparse phase runs ONCE per layer:
        hagg_raw[d] = sum_{(s,d)} dinv_s * h[s]      (0/1 selection matmuls)
    and the dense phase applies, per node chunk (128 rows):
        P_e   = hagg_raw @ W_e + sqrt(deg) x b_e     (rank-1 bias, PE k=1 mm)
        out_e = relu(P_e * (gate_e * dinv_d^p))      (ACT per-partition scale)
    where p=2 for layer 1 (folds the pre-scale of the next layer's gather
    source: we exchange hs1 = dinv*h1) and p=1 for layer 2.

Sharding: nodes are relabeled so each core owns 6250 nodes arranged into 49
windows of 128 "slots"; relabeling greedily balances sum(deg) per window.
Each core aggregates the in-edges of its own nodes, gathering source rows
(bf16) with dma_gather from a replicated source (layer 1: x*dinv from host;
layer 2: AllGather'd hs1).  Selection matrices are 0/1 fp8 streamed from HBM;
the PE contracts 128-token chunks against them into PSUM windows.
"""

import os
import numpy as np
import ml_dtypes

N = 50000
E = 800000
F = 128
HID = 128
OUT = 64
TOP = 4
EXP = 3
G = 64
TEMP = 101.0
W_CORES = 8
NSH = N // W_CORES          # 6250 nodes per core
WPC = 49                    # windows per core (48*128 + 106)
WSLOT = 128
NPAD = WPC * WSLOT          # 6272 padded local nodes
BASE_B = 17232              # second gather base (reach 17232..49999 via int16)
A_LIM = 32768               # bucket-A rows [0, 32768)
GROUPS = [(w, w + 1) for w in range(0, 48, 2)] + [(48,)]


# ----------------------------------------------------------------- host plan


def _build_plan(edge_index, batch):
    src = np.asarray(edge_index[0], dtype=np.int64)
    dst = np.asarray(edge_index[1], dtype=np.int64)
    sl = np.arange(N, dtype=np.int64)
    s_all = np.concatenate([src, sl])
    d_all = np.concatenate([dst, sl])
    deg = np.bincount(d_all, minlength=N).astype(np.float64)  # includes self
    dinv = 1.0 / np.sqrt(deg)

    # --- relabel: greedy balance of sum(deg) over 8*49 windows (cap 128/106)
    order = np.argsort(-deg, kind="stable")
    nbins = W_CORES * WPC
    caps = np.full(nbins, WSLOT, np.int64)
    caps[WPC - 1 :: WPC] = NSH - 48 * WSLOT  # last window per core: 106
    load = np.zeros(nbins, np.float64)
    fill = np.zeros(nbins, np.int64)
    import heapq

    heap = [(0.0, int(b)) for b in range(nbins)]
    heapq.heapify(heap)
    binof = np.empty(N, np.int64)
    posof = np.empty(N, np.int64)
    for nid in order:
        while True:
            l, b = heapq.heappop(heap)
            if fill[b] < caps[b]:
                break
        binof[nid] = b
        posof[nid] = fill[b]
        fill[b] += 1
        load[b] = l + deg[nid]
        if fill[b] < caps[b]:
            heapq.heappush(heap, (load[b], b))
    c_of_bin = binof // WPC
    w_of_bin = binof % WPC
    new_id = c_of_bin * NSH + w_of_bin * WSLOT + posof
    # note: within the last window pos < 106 so new ids stay inside the shard

    ns = new_id[s_all]
    nd = new_id[d_all]
    core = nd // NSH
    loc = nd % NSH
    win = loc // WSLOT
    slot = loc % WSLOT

    # per (core, window) token lists, bucketed by source row range
    RA = np.zeros(WPC, np.int64)
    RB = np.zeros(WPC, np.int64)
    tokA = {}
    tokB = {}
    okey = core * WPC + win
    osort = np.argsort(okey, kind="stable")
    ns_s, slot_s, okey_s = ns[osort], slot[osort], okey[osort]
    bounds = np.searchsorted(okey_s, np.arange(W_CORES * WPC + 1))
    nAf = np.zeros((W_CORES, WPC), np.int64)
    for c in range(W_CORES):
        for w in range(WPC):
            k = c * WPC + w
            s_cw = ns_s[bounds[k] : bounds[k + 1]]
            sl_cw = slot_s[bounds[k] : bounds[k + 1]]
            fa = s_cw < BASE_B
            fb = s_cw >= A_LIM
            fr = ~fa & ~fb
            tokA[(c, w)] = [(s_cw[fa], sl_cw[fa]), (s_cw[fr], sl_cw[fr])]
            tokB[(c, w)] = (s_cw[fb], sl_cw[fb])
            nAf[c, w] = int(fa.sum())
    for w in range(WPC):
        RA[w] = max(1, int(np.ceil(nAf[:, w].max() / WSLOT)))
    # fill A up to RA*128 with free tokens, remainder goes to B
    nB = np.zeros((W_CORES, WPC), np.int64)
    for c in range(W_CORES):
        for w in range(WPC):
            (sa, la), (sf, lf) = tokA[(c, w)]
            sb, lb = tokB[(c, w)]
            room = RA[w] * WSLOT - len(sa)
            take = min(room, len(sf))
            tokA[(c, w)] = (
                np.concatenate([sa, sf[:take]]),
                np.concatenate([la, lf[:take]]),
            )
            tokB[(c, w)] = (
                np.concatenate([sb, sf[take:]]),
                np.concatenate([lb, lf[take:]]),
            )
            nB[c, w] = len(tokB[(c, w)][0])
    for w in range(WPC):
        RB[w] = max(1, int(np.ceil(nB[:, w].max() / WSLOT)))

    # chunk storage order: per group g: [w0 A][w1 A] | [w0 B][w1 B]
    totch = int(sum((RA[w] + RB[w]) for w in range(WPC)))
    idx_np = np.zeros((W_CORES, 128, totch * 8), np.int16)
    sel_np = np.zeros((W_CORES, totch, 128, 128), ml_dtypes.float8_e4m3)
    tok_src = np.full((W_CORES, totch, 128), -1, np.int64)
    ch_base_A = {}
    ch_base_B = {}
    ch = 0
    for grp in GROUPS:
        for w in grp:
            ch_base_A[w] = ch
            ch += int(RA[w])
        for w in grp:
            ch_base_B[w] = ch
            ch += int(RB[w])
    assert ch == totch

    def fill_tokens(c, w, ch0, nch, s_arr, l_arr, base):
        n = len(s_arr)
        assert n <= nch * WSLOT
        iv = (s_arr - base).astype(np.int16)
        t = np.arange(n)
        chv = ch0 + t // WSLOT
        pv = t % WSLOT
        sel_np[c, chv, pv, l_arr] = 1.0
        tok_src[c, chv, pv] = s_arr
        # idx wrapped layout per chunk: token p at [p%16, chunk*8 + p//16]
        cols = chv * 8 + pv // 16
        rows = pv % 16
        idx_np[c, rows, cols] = iv

    for c in range(W_CORES):
        for w in range(WPC):
            sa, la = tokA[(c, w)]
            fill_tokens(c, w, ch_base_A[w], int(RA[w]), sa, la, 0)
            sb, lb = tokB[(c, w)]
            fill_tokens(c, w, ch_base_B[w], int(RB[w]), sb, lb, BASE_B)
    # replicate idx pattern across the 8 groups of 16 partitions
    idx_np[:, 16:, :] = np.tile(idx_np[:, :16, :], (1, 7, 1))

    # per-core node-level arrays in relabeled order
    inv = np.empty(N, np.int64)
    inv[new_id] = np.arange(N)

    nb = np.asarray(batch, dtype=np.int64)
    cnt = np.bincount(nb, minlength=G).astype(np.float64)

    plan = {
        "deg": deg,
        "dinv": dinv,
        "new_id": new_id,
        "inv": inv,
        "RA": RA,
        "RB": RB,
        "totch": totch,
        "idx": idx_np,
        "sel": sel_np,
        "cnt": cnt,
        "batch_new": nb[inv],  # graph id per relabeled node
        "tok_src": tok_src,
    }
    return plan


# ------------------------------------------------------------- device build


class _StageDone(Exception):
    pass


def _build_nc(RA, RB, totch, stage="full"):
    import concourse.bacc as bacc
    import concourse.mybir as mybir
    import concourse.tile as tile
    from concourse.masks import make_identity

    fp32 = mybir.dt.float32
    bf16 = mybir.dt.bfloat16
    fp8 = mybir.dt.float8e4
    i16 = mybir.dt.int16

    nc = bacc.Bacc("TRN2", debug=False, num_swdge_queues=4)

    src0 = nc.dram_tensor("src0", [N, F], bf16, kind="ExternalInput")
    tok0 = nc.dram_tensor("tok0", [totch, 128, F], bf16, kind="ExternalInput")
    idxs = nc.dram_tensor("idxs", [128, totch * 8], i16, kind="ExternalInput")
    sels = nc.dram_tensor("sels", [totch, 128, 128], fp8, kind="ExternalInput")
    wall0 = nc.dram_tensor("wall0", [F, EXP * HID], bf16, kind="ExternalInput")
    wall1 = nc.dram_tensor("wall1", [F, EXP * HID], bf16, kind="ExternalInput")
    ball0 = nc.dram_tensor("ball0", [1, EXP * HID], bf16, kind="ExternalInput")
    ball1 = nc.dram_tensor("ball1", [1, EXP * HID], bf16, kind="ExternalInput")
    sqdeg = nc.dram_tensor("sqdeg", [1, NPAD], bf16, kind="ExternalInput")
    dpow1 = nc.dram_tensor("dpow1", [128, WPC], fp32, kind="ExternalInput")
    dpow2 = nc.dram_tensor("dpow2", [128, WPC], fp32, kind="ExternalInput")
    topt = nc.dram_tensor("topt", [TOP, NPAD], fp32, kind="ExternalInput")
    wgt0 = nc.dram_tensor("wgt0", [TOP, EXP], fp32, kind="ExternalInput")
    wgt1 = nc.dram_tensor("wgt1", [TOP, EXP], fp32, kind="ExternalInput")
    selg = nc.dram_tensor("selg", [WPC, 128, G], fp32, kind="ExternalInput")
    wf = nc.dram_tensor("wf", [HID, OUT], fp32, kind="ExternalInput")
    if stage == "sparse1":
        hagg_out = nc.dram_tensor("hagg_out", [128, NPAD], fp32, kind="ExternalOutput")
    else:
        pout = nc.dram_tensor("pout", [G, OUT], fp32, kind="ExternalOutput")

    shard = nc.dram_tensor("shard_hs1", [NSH, F], bf16)
    full1 = nc.dram_tensor("full_hs1", [N, F], bf16, addr_space="Shared")

    with tile.TileContext(nc) as tc:
        with tc.tile_pool(name="persist", bufs=1) as pp, \
             tc.tile_pool(name="wt", bufs=1) as wtp, \
             tc.tile_pool(name="stream", bufs=2) as sp, \
             tc.tile_pool(name="chunks", bufs=3) as cp, \
             tc.tile_pool(name="psum", bufs=3, space="PSUM") as ps, \
             tc.tile_pool(name="psume", bufs=2, space="PSUM") as pse, \
             tc.tile_pool(name="psump", bufs=1, space="PSUM") as psp:

            # ---------- resident data
            idx_sb = pp.tile([128, totch * 8], i16)
            nc.sync.dma_start(out=idx_sb[:], in_=idxs[:])
            hagg = pp.tile([128, NPAD], bf16)          # haggT, f-major
            sq_sb = pp.tile([1, NPAD], bf16)
            nc.sync.dma_start(out=sq_sb[:], in_=sqdeg[:])
            topt_sb = pp.tile([TOP, NPAD], fp32)
            nc.sync.dma_start(out=topt_sb[:], in_=topt[:])
            ident = pp.tile([128, 128], fp32)
            make_identity(nc, ident[:])
            w_sb = [wtp.tile([F, EXP * HID], bf16, tag=f"w{l}", name=f"w{l}") for l in range(2)]
            nc.sync.dma_start(out=w_sb[0][:], in_=wall0[:])
            nc.sync.dma_start(out=w_sb[1][:], in_=wall1[:])
            b_sb = [wtp.tile([1, EXP * HID], bf16, tag=f"b{l}", name=f"b{l}") for l in range(2)]
            nc.sync.dma_start(out=b_sb[0][:], in_=ball0[:])
            nc.sync.dma_start(out=b_sb[1][:], in_=ball1[:])
            wg_sb = [wtp.tile([TOP, EXP], fp32, tag=f"wg{l}", name=f"wg{l}") for l in range(2)]
            nc.sync.dma_start(out=wg_sb[0][:], in_=wgt0[:])
            nc.sync.dma_start(out=wg_sb[1][:], in_=wgt1[:])
            dp_sb = [wtp.tile([128, WPC], fp32, tag=f"dp{l}", name=f"dp{l}") for l in range(2)]
            nc.sync.dma_start(out=dp_sb[0][:], in_=dpow1[:])
            nc.sync.dma_start(out=dp_sb[1][:], in_=dpow2[:])
            wf_sb = wtp.tile([HID, OUT], fp32)
            nc.sync.dma_start(out=wf_sb[:], in_=wf[:])

            # ---------- gate scale columns for both layers: [128, WPC, EXP]
            do_gates = os.environ.get("KERNEL_GATES", "1") == "1"
            scale_sb = []
            for l in range(2 if do_gates else 0):
                glog = pp.tile([128, WPC, EXP], fp32, tag=f"glog{l}", name=f"glog{l}")
                for k in range(WPC):
                    pg = ps.tile([128, EXP], fp32, space="PSUM", tag="pw")
                    nc.tensor.matmul(
                        out=pg[:],
                        lhsT=topt_sb[:, k * 128 : (k + 1) * 128],
                        rhs=wg_sb[l][:],
                        start=True, stop=True,
                    )
                    nc.vector.tensor_copy(out=glog[:, k, :], in_=pg[:])
                gexp = pp.tile([128, WPC, EXP], fp32, tag=f"gexp{l}", name=f"gexp{l}")
                nc.scalar.activation(gexp[:], glog[:],
                                     mybir.ActivationFunctionType.Exp,
                                     bias=0.0, scale=1.0 / TEMP)
                gsum = pp.tile([128, WPC], fp32, tag=f"gsum{l}", name=f"gsum{l}")
                nc.vector.tensor_reduce(out=gsum[:], in_=gexp[:],
                                        axis=mybir.AxisListType.X,
                                        op=mybir.AluOpType.add)
                grec = pp.tile([128, WPC], fp32, tag=f"grec{l}", name=f"grec{l}")
                nc.vector.reciprocal(out=grec[:], in_=gsum[:])
                rd = pp.tile([128, WPC], fp32, tag=f"rd{l}", name=f"rd{l}")
                nc.vector.tensor_mul(out=rd[:], in0=grec[:], in1=dp_sb[l][:])
                sc = pp.tile([128, WPC, EXP], fp32, tag=f"sc{l}", name=f"sc{l}")
                for e in range(EXP):
                    nc.vector.tensor_mul(out=sc[:, :, e], in0=gexp[:, :, e],
                                         in1=rd[:])
                scale_sb.append(sc)

            # chunk offsets in storage/call order
            chA, chB = {}, {}
            ch = 0
            for grp in GROUPS:
                for w in grp:
                    chA[w] = ch
                    ch += int(RA[w])
                for w in grp:
                    chB[w] = ch
                    ch += int(RB[w])

            ppool = psp.tile([G, HID], fp32, space="PSUM")

            ngroups = int(os.environ.get("KERNEL_NGROUPS", "25"))
            do_dense = os.environ.get("KERNEL_DENSE", "1") == "1"

            qrr = [0]

            def gather_split(gtile, src_ap, ch0, nch, goff):
                # subcalls of <=8 chunks (1024 idxs) so single_packet fits
                off = 0
                while off < nch:
                    n = min(8, nch - off)
                    nc.gpsimd.dma_gather(
                        gtile[:, goff + off : goff + off + n, :], src_ap,
                        idx_sb[:, (ch0 + off) * 8 : (ch0 + off + n) * 8],
                        n * 128, n * 128, F, single_packet=True,
                        queue_num=qrr[0] % 4)
                    qrr[0] += 1
                    off += n

            def sparse_and_dense(l, source, store_l1):
                for grp in GROUPS[:ngroups]:
                    ra = sum(int(RA[w]) for w in grp)
                    rb = sum(int(RB[w]) for w in grp)
                    c0 = chA[grp[0]]
                    cb0 = chB[grp[0]]
                    if l == 0:
                        gAll = cp.tile([128, ra + rb, F], bf16, tag="gAll")
                        nc.sync.dma_start(
                            out=gAll[:],
                            in_=tok0[c0 : c0 + ra + rb].rearrange(
                                "r p f -> p r f"))
                        gA = gAll[:, 0:ra, :]
                        gB = gAll[:, ra : ra + rb, :]
                    else:
                        gAt = cp.tile([128, ra, F], bf16, tag="gA")
                        gather_split(gAt, source[0:N, :], c0, ra, 0)
                        gBt = cp.tile([128, rb, F], bf16, tag="gB")
                        gather_split(gBt, source[BASE_B:N, :], cb0, rb, 0)
                        gA, gB = gAt[:], gBt[:]
                    selA = cp.tile([128, ra, 128], fp8, tag="selA")
                    nc.sync.dma_start(
                        out=selA[:],
                        in_=sels[c0 : c0 + ra].rearrange("r p c -> p r c"))
                    selB = cp.tile([128, rb, 128], fp8, tag="selB")
                    nc.sync.dma_start(
                        out=selB[:],
                        in_=sels[cb0 : cb0 + rb].rearrange("r p c -> p r c"))
                    a_off = 0
                    b_off = 0
                    for w in grp:
                        pw = ps.tile([128, WSLOT], fp32, space="PSUM", tag="pw")
                        nchw = int(RA[w]) + int(RB[w])
                        j = 0
                        for r in range(int(RA[w])):
                            nc.tensor.matmul(
                                out=pw[:],
                                lhsT=gA[:, a_off + r, :],
                                rhs=selA[:, a_off + r, :],
                                start=(j == 0), stop=(j == nchw - 1))
                            j += 1
                        for r in range(int(RB[w])):
                            nc.tensor.matmul(
                                out=pw[:],
                                lhsT=gB[:, b_off + r, :],
                                rhs=selB[:, b_off + r, :],
                                start=(j == 0), stop=(j == nchw - 1))
                            j += 1
                        a_off += int(RA[w])
                        b_off += int(RB[w])
                        nc.vector.tensor_copy(
                            out=hagg[:, w * 128 : (w + 1) * 128], in_=pw[:])

                # dense phase, per node chunk
                for k in range(WPC if do_dense else 0):
                    pe = pse.tile([128, EXP * HID], fp32, space="PSUM", tag="pe")
                    nc.tensor.matmul(
                        out=pe[:], lhsT=hagg[:, k * 128 : (k + 1) * 128],
                        rhs=w_sb[l][:], start=True, stop=False)
                    nc.tensor.matmul(
                        out=pe[:], lhsT=sq_sb[:, k * 128 : (k + 1) * 128],
                        rhs=b_sb[l][:], start=False, stop=True)
                    aex = []
                    for e in range(EXP):
                        if not do_gates:
                            break
                        a = cp.tile([128, HID], fp32, tag=f"a{e}", name=f"a{e}")
                        nc.scalar.activation(
                            a[:], pe[:, e * HID : (e + 1) * HID],
                            mybir.ActivationFunctionType.Relu,
                            bias=0.0, scale=scale_sb[l][:, k, e : e + 1])
                        aex.append(a)
                    hout = cp.tile([128, HID], fp32, tag="hout")
                    nc.vector.tensor_add(out=hout[:], in0=aex[0][:], in1=aex[1][:])
                    nc.vector.tensor_add(out=hout[:], in0=hout[:], in1=aex[2][:])
                    rows = min(128, NSH - k * 128)
                    if store_l1:
                        hbf = cp.tile([128, HID], bf16, tag="hbf")
                        nc.vector.tensor_copy(out=hbf[:], in_=hout[:])
                        nc.sync.dma_start(
                            out=shard[k * 128 : k * 128 + rows, :],
                            in_=hbf[:rows, :])
                    else:
                        sg = cp.tile([128, G], fp32, tag="sg")
                        nc.sync.dma_start(out=sg[:], in_=selg[k])
                        nc.tensor.matmul(
                            out=ppool[:], lhsT=sg[:], rhs=hout[:],
                            start=(k == 0), stop=(k == WPC - 1))

            # ---------- layer 1
            sparse_and_dense(0, src0, store_l1=(stage != "sparse1"))
            if stage == "sparse1":
                nc.sync.dma_start(out=hagg_out[:], in_=hagg[:])
            if stage == "full":
                nc.gpsimd.collective_compute(
                    "AllGather", mybir.AluOpType.bypass,
                    ins=[shard[:]], outs=[full1[:]],
                    replica_groups=[list(range(W_CORES))])
                # ---------- layer 2
                sparse_and_dense(1, full1, store_l1=False)

                # ---------- pooled @ Wf
                pooled = sp.tile([G, HID], fp32)
                nc.vector.tensor_copy(out=pooled[:], in_=ppool[:])
                ptr = ps.tile([128, G], fp32, space="PSUM", tag="pw")
                nc.tensor.transpose(out=ptr[:, :G], in_=pooled[:],
                                    identity=ident[:G, :G])
                pooledT = sp.tile([HID, G], fp32)
                nc.vector.tensor_copy(out=pooledT[:], in_=ptr[:, :G])
                pfin = ps.tile([G, OUT], fp32, space="PSUM", tag="pw")
                nc.tensor.matmul(out=pfin[:], lhsT=pooledT[:], rhs=wf_sb[:],
                                 start=True, stop=True)
                ofin = sp.tile([G, OUT], fp32)
                nc.vector.tensor_copy(out=ofin[:], in_=pfin[:])
                nc.sync.dma_start(out=pout[:], in_=ofin[:])
            elif stage == "layer1":
                nc.sync.dma_start(out=pout[:], in_=ident[:G, :OUT])

    nc.compile()
    return nc


# ------------------------------------------------------------------- kernel


def kernel(**inputs):
    x = np.asarray(inputs["x"], np.float32)
    top_features = np.asarray(inputs["top_features"], np.float32)
    edge_index = np.asarray(inputs["edge_index"])
    batch = np.asarray(inputs["batch"])
    W0 = np.asarray(inputs["W0"], np.float32)
    b0 = np.asarray(inputs["b0"], np.float32)
    Wg0 = np.asarray(inputs["Wg0"], np.float32)
    W1 = np.asarray(inputs["W1"], np.float32)
    b1 = np.asarray(inputs["b1"], np.float32)
    Wg1 = np.asarray(inputs["Wg1"], np.float32)
    Wf = np.asarray(inputs["Wf"], np.float32)
    bf = np.asarray(inputs["bf"], np.float32)

    plan = _build_plan(edge_index, batch)
    dinv = plan["dinv"]
    inv = plan["inv"]          # relabeled -> original node id
    RA, RB, totch = plan["RA"], plan["RB"], plan["totch"]

    # gather source (layer 1): x * dinv, relabeled order, bf16
    xs = (x * dinv[:, None])[inv].astype(ml_dtypes.bfloat16)

    deg_new = plan["deg"][inv]
    dinv_new = dinv[inv]
    top_new = top_features[inv]
    batch_new = plan["batch_new"]

    def pad_npad(a):
        out = np.zeros((W_CORES, NPAD) + a.shape[1:], a.dtype)
        for c in range(W_CORES):
            out[c, : 48 * WSLOT] = a[c * NSH : c * NSH + 48 * WSLOT]
            # last window: 106 real slots
            out[c, 48 * WSLOT : 48 * WSLOT + (NSH - 48 * WSLOT)] = \
                a[c * NSH + 48 * WSLOT : (c + 1) * NSH]
        return out

    sq_pad = pad_npad(np.sqrt(deg_new).astype(np.float32))       # [8, NPAD]
    d1_pad = pad_npad((dinv_new ** 2).astype(np.float32))
    d2_pad = pad_npad(dinv_new.astype(np.float32))
    top_pad = pad_npad(top_new.astype(np.float32))               # [8,NPAD,4]
    bat_pad = pad_npad(batch_new)
    # mark pad slots: zero scales, selg zero
    padmask = pad_npad(np.ones(N, np.float32))

    d1_pad *= padmask
    d2_pad *= padmask

    wall0 = W0.transpose(1, 0, 2).reshape(F, EXP * HID).copy()
    wall1 = W1.transpose(1, 0, 2).reshape(F, EXP * HID).copy()
    ball0 = b0.reshape(1, EXP * HID).copy()
    ball1 = b1.reshape(1, EXP * HID).copy()

    in_maps = []
    for c in range(W_CORES):
        selg_c = np.zeros((WPC, 128, G), np.float32)
        bm = bat_pad[c].reshape(WPC, 128)
        pm = padmask[c].reshape(WPC, 128)
        wv, pv = np.nonzero(pm > 0)
        selg_c[wv, pv, bm[wv, pv]] = 1.0
        ts = plan["tok_src"][c]
        tok0_c = np.zeros((ts.shape[0], 128, F), ml_dtypes.bfloat16)
        valid = ts >= 0
        tok0_c[valid] = xs[ts[valid]]
        in_maps.append({
            "src0": xs,
            "tok0": tok0_c,
            "idxs": plan["idx"][c],
            "sels": plan["sel"][c],
            "wall0": wall0.astype(ml_dtypes.bfloat16),
            "wall1": wall1.astype(ml_dtypes.bfloat16),
            "ball0": ball0.astype(ml_dtypes.bfloat16),
            "ball1": ball1.astype(ml_dtypes.bfloat16),
            "sqdeg": sq_pad[c][None, :].astype(ml_dtypes.bfloat16),
            "dpow1": d1_pad[c].reshape(WPC, 128).T.copy(),
            "dpow2": d2_pad[c].reshape(WPC, 128).T.copy(),
            "topt": top_pad[c].T.copy(),
            "wgt0": Wg0.T.copy(), "wgt1": Wg1.T.copy(),
            "selg": selg_c,
            "wf": Wf.copy(),
        })

    from concourse.bass_utils import run_bass_kernel_spmd

    stage = os.environ.get("KERNEL_STAGE", "full")
    nc = _build_nc(RA, RB, totch, stage=stage)
    trace = os.environ.get("KERNEL_TRACE", "0") == "1"
    ncores = int(os.environ.get("KERNEL_CORES", str(W_CORES)))
    res = run_bass_kernel_spmd(nc, in_maps[:ncores], core_ids=list(range(ncores)),
                               trace=trace)
    kernel.last_results = res

    total = np.zeros((G, OUT), np.float64)
    for c in range(W_CORES):
        total += res.results[c]["pout"].astype(np.float64)
    cnt = np.maximum(plan["cnt"], 1.0)
    out = total / cnt[:, None] + bf.astype(np.float64)[None, :]
    return out.astype(np.float32)



# revision 8
# speedup vs baseline: 1.3003x; 1.3003x over previous
"""CAMoE-GNN Trainium2 kernel (8 NeuronCores, SPMD).

Math (reference, per layer):
    gate = softmax((top @ Wg.T)/TEMP)            [N,3]
    he   = h @ W[e]
    agg  = segsum(he[src]*dinv[src]*dinv[dst] -> dst)   (incl. self loops)
    out  = sum_e gate_e * relu(agg_e + b[e])

Key algebra used here:
    aggregation commutes with W[e]:  agg_e = (A @ h) @ W[e]  with
    A = D^-1/2 (M + I) D^-1/2.  So the sparse phase runs ONCE per layer:
        hagg_raw[d] = sum_{(s,d)} dinv_s * h[s]      (0/1 selection matmuls)
    and the dense phase applies, per node chunk (128 rows):
        P_e   = hagg_raw @ W_e + sqrt(deg) x b_e     (rank-1 bias, PE k=1 mm)
        out_e = relu(P_e * (gate_e * dinv_d^p))      (ACT per-partition scale)
    where p=2 for layer 1 (folds the pre-scale of the next layer's gather
    source: we exchange hs1 = dinv*h1) and p=1 for layer 2.
    The gate/dinv scale columns are tiny and computed on host.

Sharding: nodes are relabeled so each core owns 6250 nodes arranged into 49
windows of 128 "slots"; relabeling greedily balances sum(deg) per window.
Each core aggregates the in-edges of its own nodes.  Layer 1 token features
(x*dinv rows per edge) are pre-gathered on host and STREAMED from HBM in
partition-major layout (contiguous ~8KB per partition per group); layer 2
gathers source rows (bf16) with dma_gather from the AllGather'd hs1.
Selection matrices are 0/1 fp8, streamed once (partition-major) into a
resident SBUF tile during layer 1 and reused by layer 2.
"""

import os
import numpy as np
import ml_dtypes

N = 50000
E = 800000
F = 128
HID = 128
OUT = 64
TOP = 4
EXP = 3
G = 64
TEMP = 101.0
W_CORES = 8
NSH = N // W_CORES          # 6250 nodes per core
WPC = 49                    # windows per core (48*128 + 106)
WSLOT = 128
NPAD = WPC * WSLOT          # 6272 padded local nodes
BASE_B = 17232              # second gather base (reach 17232..49999 via int16)
A_LIM = 32768               # bucket-A rows [0, 32768)
GROUPS = [(w, w + 1) for w in range(0, 48, 2)] + [(48,)]


# ----------------------------------------------------------------- host plan


def _build_plan(edge_index, batch):
    src = np.asarray(edge_index[0], dtype=np.int64)
    dst = np.asarray(edge_index[1], dtype=np.int64)
    sl = np.arange(N, dtype=np.int64)
    s_all = np.concatenate([src, sl])
    d_all = np.concatenate([dst, sl])
    deg = np.bincount(d_all, minlength=N).astype(np.float64)  # includes self
    dinv = 1.0 / np.sqrt(deg)

    # --- relabel: greedy balance of sum(deg) over 8*49 windows (cap 128/106)
    order = np.argsort(-deg, kind="stable")
    nbins = W_CORES * WPC
    caps = np.full(nbins, WSLOT, np.int64)
    caps[WPC - 1 :: WPC] = NSH - 48 * WSLOT  # last window per core: 106
    load = np.zeros(nbins, np.float64)
    fill = np.zeros(nbins, np.int64)
    import heapq

    heap = [(0.0, int(b)) for b in range(nbins)]
    heapq.heapify(heap)
    binof = np.empty(N, np.int64)
    posof = np.empty(N, np.int64)
    for nid in order:
        while True:
            l, b = heapq.heappop(heap)
            if fill[b] < caps[b]:
                break
        binof[nid] = b
        posof[nid] = fill[b]
        fill[b] += 1
        load[b] = l + deg[nid]
        if fill[b] < caps[b]:
            heapq.heappush(heap, (load[b], b))
    c_of_bin = binof // WPC
    w_of_bin = binof % WPC
    new_id = c_of_bin * NSH + w_of_bin * WSLOT + posof
    # note: within the last window pos < 106 so new ids stay inside the shard

    ns = new_id[s_all]
    nd = new_id[d_all]
    core = nd // NSH
    loc = nd % NSH
    win = loc // WSLOT
    slot = loc % WSLOT

    # per (core, window) token lists, bucketed by source row range
    RA = np.zeros(WPC, np.int64)
    RB = np.zeros(WPC, np.int64)
    tokA = {}
    tokB = {}
    okey = core * WPC + win
    osort = np.argsort(okey, kind="stable")
    ns_s, slot_s, okey_s = ns[osort], slot[osort], okey[osort]
    bounds = np.searchsorted(okey_s, np.arange(W_CORES * WPC + 1))
    nAf = np.zeros((W_CORES, WPC), np.int64)
    for c in range(W_CORES):
        for w in range(WPC):
            k = c * WPC + w
            s_cw = ns_s[bounds[k] : bounds[k + 1]]
            sl_cw = slot_s[bounds[k] : bounds[k + 1]]
            fa = s_cw < BASE_B
            fb = s_cw >= A_LIM
            fr = ~fa & ~fb
            tokA[(c, w)] = [(s_cw[fa], sl_cw[fa]), (s_cw[fr], sl_cw[fr])]
            tokB[(c, w)] = (s_cw[fb], sl_cw[fb])
            nAf[c, w] = int(fa.sum())
    for w in range(WPC):
        RA[w] = max(1, int(np.ceil(nAf[:, w].max() / WSLOT)))
    # fill A up to RA*128 with free tokens, remainder goes to B
    nB = np.zeros((W_CORES, WPC), np.int64)
    for c in range(W_CORES):
        for w in range(WPC):
            (sa, la), (sf, lf) = tokA[(c, w)]
            sb, lb = tokB[(c, w)]
            room = RA[w] * WSLOT - len(sa)
            take = min(room, len(sf))
            tokA[(c, w)] = (
                np.concatenate([sa, sf[:take]]),
                np.concatenate([la, lf[:take]]),
            )
            tokB[(c, w)] = (
                np.concatenate([sb, sf[take:]]),
                np.concatenate([lb, lf[take:]]),
            )
            nB[c, w] = len(tokB[(c, w)][0])
    for w in range(WPC):
        RB[w] = max(1, int(np.ceil(nB[:, w].max() / WSLOT)))

    # chunk storage order: per group g: [w0 A][w1 A] | [w0 B][w1 B]
    totch = int(sum((RA[w] + RB[w]) for w in range(WPC)))
    idx_np = np.zeros((W_CORES, 128, totch * 8), np.int16)
    sel_np = np.zeros((W_CORES, 128, totch, 128), ml_dtypes.float8_e4m3)
    tok_src = np.full((W_CORES, totch, 128), -1, np.int64)
    ch_base_A = {}
    ch_base_B = {}
    ch = 0
    for grp in GROUPS:
        for w in grp:
            ch_base_A[w] = ch
            ch += int(RA[w])
        for w in grp:
            ch_base_B[w] = ch
            ch += int(RB[w])
    assert ch == totch

    def fill_tokens(c, w, ch0, nch, s_arr, l_arr, base):
        n = len(s_arr)
        assert n <= nch * WSLOT
        iv = (s_arr - base).astype(np.int16)
        t = np.arange(n)
        chv = ch0 + t // WSLOT
        pv = t % WSLOT
        sel_np[c, pv, chv, l_arr] = 1.0
        tok_src[c, chv, pv] = s_arr
        # idx wrapped layout per chunk: token p at [p%16, chunk*8 + p//16]
        cols = chv * 8 + pv // 16
        rows = pv % 16
        idx_np[c, rows, cols] = iv

    for c in range(W_CORES):
        for w in range(WPC):
            sa, la = tokA[(c, w)]
            fill_tokens(c, w, ch_base_A[w], int(RA[w]), sa, la, 0)
            sb, lb = tokB[(c, w)]
            fill_tokens(c, w, ch_base_B[w], int(RB[w]), sb, lb, BASE_B)
    # replicate idx pattern across the 8 groups of 16 partitions
    idx_np[:, 16:, :] = np.tile(idx_np[:, :16, :], (1, 7, 1))

    # per-core node-level arrays in relabeled order
    inv = np.empty(N, np.int64)
    inv[new_id] = np.arange(N)

    nb = np.asarray(batch, dtype=np.int64)
    cnt = np.bincount(nb, minlength=G).astype(np.float64)

    plan = {
        "deg": deg,
        "dinv": dinv,
        "new_id": new_id,
        "inv": inv,
        "RA": RA,
        "RB": RB,
        "totch": totch,
        "idx": idx_np,
        "sel": sel_np,
        "cnt": cnt,
        "batch_new": nb[inv],  # graph id per relabeled node
        "tok_src": tok_src,
    }
    return plan


# ------------------------------------------------------------- device build


def _build_nc(RA, RB, totch):
    import concourse.bacc as bacc
    import concourse.mybir as mybir
    import concourse.tile as tile
    from concourse.masks import make_identity

    fp32 = mybir.dt.float32
    bf16 = mybir.dt.bfloat16
    fp8 = mybir.dt.float8e4
    i16 = mybir.dt.int16

    nc = bacc.Bacc("TRN2", debug=False, num_swdge_queues=4)

    tok0 = nc.dram_tensor("tok0", [128, totch, F], bf16, kind="ExternalInput")
    idxs = nc.dram_tensor("idxs", [128, totch * 8], i16, kind="ExternalInput")
    sels = nc.dram_tensor("sels", [128, totch * 128], fp8, kind="ExternalInput")
    wall0 = nc.dram_tensor("wall0", [F, EXP * HID], bf16, kind="ExternalInput")
    wall1 = nc.dram_tensor("wall1", [F, EXP * HID], bf16, kind="ExternalInput")
    ball0 = nc.dram_tensor("ball0", [1, EXP * HID], bf16, kind="ExternalInput")
    ball1 = nc.dram_tensor("ball1", [1, EXP * HID], bf16, kind="ExternalInput")
    sqdeg = nc.dram_tensor("sqdeg", [1, NPAD], bf16, kind="ExternalInput")
    scal0 = nc.dram_tensor("scal0", [128, WPC * EXP], fp32, kind="ExternalInput")
    scal1 = nc.dram_tensor("scal1", [128, WPC * EXP], fp32, kind="ExternalInput")
    selg = nc.dram_tensor("selg", [128, WPC * G], bf16, kind="ExternalInput")
    wf = nc.dram_tensor("wf", [HID, OUT], fp32, kind="ExternalInput")
    pout = nc.dram_tensor("pout", [G, OUT], fp32, kind="ExternalOutput")

    shard = nc.dram_tensor("shard_hs1", [NSH, F], bf16)
    full1 = nc.dram_tensor("full_hs1", [N, F], bf16, addr_space="Shared")

    with tile.TileContext(nc) as tc:
        with tc.tile_pool(name="persist", bufs=1) as pp, \
             tc.tile_pool(name="wt", bufs=1) as wtp, \
             tc.tile_pool(name="stream", bufs=2) as sp, \
             tc.tile_pool(name="chunks", bufs=2) as cp, \
             tc.tile_pool(name="psum", bufs=3, space="PSUM") as ps, \
             tc.tile_pool(name="psume", bufs=2, space="PSUM") as pse, \
             tc.tile_pool(name="psump", bufs=1, space="PSUM") as psp:

            # ---------- resident data
            idx_sb = pp.tile([128, totch * 8], i16)
            nc.sync.dma_start(out=idx_sb[:], in_=idxs[:])
            hagg = pp.tile([128, NPAD], bf16)          # haggT, f-major
            sel_sb = pp.tile([128, totch * 128], fp8)  # resident 0/1 matrices
            selg_sb = pp.tile([128, WPC * G], bf16)
            nc.sync.dma_start(out=selg_sb[:], in_=selg[:])
            sq_sb = pp.tile([1, NPAD], bf16)
            nc.sync.dma_start(out=sq_sb[:], in_=sqdeg[:])
            w_sb = [wtp.tile([F, EXP * HID], bf16, tag=f"w{l}", name=f"w{l}") for l in range(2)]
            nc.sync.dma_start(out=w_sb[0][:], in_=wall0[:])
            nc.sync.dma_start(out=w_sb[1][:], in_=wall1[:])
            b_sb = [wtp.tile([1, EXP * HID], bf16, tag=f"b{l}", name=f"b{l}") for l in range(2)]
            nc.sync.dma_start(out=b_sb[0][:], in_=ball0[:])
            nc.sync.dma_start(out=b_sb[1][:], in_=ball1[:])
            sc_sb = [wtp.tile([128, WPC * EXP], fp32, tag=f"sc{l}", name=f"sc{l}") for l in range(2)]
            nc.sync.dma_start(out=sc_sb[0][:], in_=scal0[:])
            nc.sync.dma_start(out=sc_sb[1][:], in_=scal1[:])
            wf_sb = wtp.tile([HID, OUT], fp32)
            nc.sync.dma_start(out=wf_sb[:], in_=wf[:])

            # chunk offsets in storage/call order
            chA, chB = {}, {}
            ch = 0
            for grp in GROUPS:
                for w in grp:
                    chA[w] = ch
                    ch += int(RA[w])
                for w in grp:
                    chB[w] = ch
                    ch += int(RB[w])

            ppool = psp.tile([G, HID], fp32, space="PSUM")

            ngroups = int(os.environ.get("KERNEL_NGROUPS", "25"))
            do_dense = os.environ.get("KERNEL_DENSE", "1") == "1"

            qrr = [0]

            def gather_split(gtile, src_ap, ch0, nch, goff):
                # subcalls of <=8 chunks (1024 idxs) so single_packet fits
                off = 0
                while off < nch:
                    n = min(8, nch - off)
                    nc.gpsimd.dma_gather(
                        gtile[:, goff + off : goff + off + n, :], src_ap,
                        idx_sb[:, (ch0 + off) * 8 : (ch0 + off + n) * 8],
                        n * 128, n * 128, F, single_packet=True,
                        queue_num=qrr[0] % 4)
                    qrr[0] += 1
                    off += n

            def sparse_and_dense(l, store_l1):
                for grp in GROUPS[:ngroups]:
                    ra = sum(int(RA[w]) for w in grp)
                    rb = sum(int(RB[w]) for w in grp)
                    c0 = chA[grp[0]]
                    cb0 = chB[grp[0]]
                    nch = ra + rb
                    if l == 0:
                        gAll = cp.tile([128, nch, F], bf16, tag="gAll")
                        nc.sync.dma_start(
                            out=gAll[:], in_=tok0[:, c0 : c0 + nch, :])
                        gA = gAll[:, 0:ra, :]
                        gB = gAll[:, ra:nch, :]
                        # stream this group's 0/1 matrices into the
                        # resident tile (contiguous per partition)
                        nc.sync.dma_start(
                            out=sel_sb[:, c0 * 128 : (c0 + nch) * 128],
                            in_=sels[:, c0 * 128 : (c0 + nch) * 128])
                    else:
                        gAt = cp.tile([128, ra, F], bf16, tag="gA")
                        gather_split(gAt, full1[0:N, :], c0, ra, 0)
                        gBt = cp.tile([128, rb, F], bf16, tag="gB")
                        gather_split(gBt, full1[BASE_B:N, :], cb0, rb, 0)
                        gA, gB = gAt[:], gBt[:]
                    a_off = 0
                    b_off = 0
                    for w in grp:
                        pw = ps.tile([128, WSLOT], fp32, space="PSUM", tag="pw")
                        nchw = int(RA[w]) + int(RB[w])
                        j = 0
                        for r in range(int(RA[w])):
                            c = chA[w] + r
                            nc.tensor.matmul(
                                out=pw[:],
                                lhsT=gA[:, a_off + r, :],
                                rhs=sel_sb[:, c * 128 : (c + 1) * 128],
                                start=(j == 0), stop=(j == nchw - 1))
                            j += 1
                        for r in range(int(RB[w])):
                            c = chB[w] + r
                            nc.tensor.matmul(
                                out=pw[:],
                                lhsT=gB[:, b_off + r, :],
                                rhs=sel_sb[:, c * 128 : (c + 1) * 128],
                                start=(j == 0), stop=(j == nchw - 1))
                            j += 1
                        a_off += int(RA[w])
                        b_off += int(RB[w])
                        nc.vector.tensor_copy(
                            out=hagg[:, w * 128 : (w + 1) * 128], in_=pw[:])

                # dense phase, per node chunk
                for k in range(WPC if do_dense else 0):
                    pe = pse.tile([128, EXP * HID], fp32, space="PSUM", tag="pe")
                    nc.tensor.matmul(
                        out=pe[:], lhsT=hagg[:, k * 128 : (k + 1) * 128],
                        rhs=w_sb[l][:], start=True, stop=False)
                    nc.tensor.matmul(
                        out=pe[:], lhsT=sq_sb[:, k * 128 : (k + 1) * 128],
                        rhs=b_sb[l][:], start=False, stop=True)
                    aex = []
                    for e in range(EXP):
                        a = cp.tile([128, HID], bf16, tag=f"a{e}", name=f"a{e}")
                        nc.scalar.activation(
                            a[:], pe[:, e * HID : (e + 1) * HID],
                            mybir.ActivationFunctionType.Relu,
                            bias=0.0,
                            scale=sc_sb[l][:, k * EXP + e : k * EXP + e + 1])
                        aex.append(a)
                    hout = cp.tile([128, HID], bf16, tag="hout")
                    nc.vector.tensor_add(out=hout[:], in0=aex[0][:], in1=aex[1][:])
                    nc.vector.tensor_add(out=hout[:], in0=hout[:], in1=aex[2][:])
                    rows = min(128, NSH - k * 128)
                    if store_l1:
                        nc.sync.dma_start(
                            out=shard[k * 128 : k * 128 + rows, :],
                            in_=hout[:rows, :])
                    else:
                        nc.tensor.matmul(
                            out=ppool[:],
                            lhsT=selg_sb[:, k * G : (k + 1) * G],
                            rhs=hout[:],
                            start=(k == 0), stop=(k == WPC - 1))

            # ---------- layer 1
            sparse_and_dense(0, store_l1=True)
            nc.gpsimd.collective_compute(
                "AllGather", mybir.AluOpType.bypass,
                ins=[shard[:]], outs=[full1[:]],
                replica_groups=[list(range(W_CORES))])
            # ---------- layer 2
            sparse_and_dense(1, store_l1=False)

            # ---------- pooled @ Wf
            pooled = sp.tile([G, HID], fp32)
            nc.vector.tensor_copy(out=pooled[:], in_=ppool[:])
            ptr = ps.tile([128, G], fp32, space="PSUM", tag="pw")
            ident = sp.tile([G, G], fp32)
            make_identity(nc, ident[:])
            nc.tensor.transpose(out=ptr[:, :G], in_=pooled[:], identity=ident[:])
            pooledT = sp.tile([HID, G], fp32)
            nc.vector.tensor_copy(out=pooledT[:], in_=ptr[:, :G])
            pfin = ps.tile([G, OUT], fp32, space="PSUM", tag="pw")
            nc.tensor.matmul(out=pfin[:], lhsT=pooledT[:], rhs=wf_sb[:],
                             start=True, stop=True)
            ofin = sp.tile([G, OUT], fp32)
            nc.vector.tensor_copy(out=ofin[:], in_=pfin[:])
            nc.sync.dma_start(out=pout[:], in_=ofin[:])

    nc.compile()
    return nc


# ------------------------------------------------------------------- kernel


def kernel(**inputs):
    x = np.asarray(inputs["x"], np.float32)
    top_features = np.asarray(inputs["top_features"], np.float32)
    edge_index = np.asarray(inputs["edge_index"])
    batch = np.asarray(inputs["batch"])
    W0 = np.asarray(inputs["W0"], np.float32)
    b0 = np.asarray(inputs["b0"], np.float32)
    Wg0 = np.asarray(inputs["Wg0"], np.float32)
    W1 = np.asarray(inputs["W1"], np.float32)
    b1 = np.asarray(inputs["b1"], np.float32)
    Wg1 = np.asarray(inputs["Wg1"], np.float32)
    Wf = np.asarray(inputs["Wf"], np.float32)
    bf = np.asarray(inputs["bf"], np.float32)

    plan = _build_plan(edge_index, batch)
    dinv = plan["dinv"]
    inv = plan["inv"]          # relabeled -> original node id
    RA, RB, totch = plan["RA"], plan["RB"], plan["totch"]

    # gather source (layer 1): x * dinv, relabeled order, bf16
    xs = (x * dinv[:, None])[inv].astype(ml_dtypes.bfloat16)

    deg_new = plan["deg"][inv]
    dinv_new = dinv[inv]
    top_new = top_features[inv]
    batch_new = plan["batch_new"]

    def pad_npad(a):
        out = np.zeros((W_CORES, NPAD) + a.shape[1:], a.dtype)
        for c in range(W_CORES):
            out[c, : 48 * WSLOT] = a[c * NSH : c * NSH + 48 * WSLOT]
            # last window: 106 real slots
            out[c, 48 * WSLOT : 48 * WSLOT + (NSH - 48 * WSLOT)] = \
                a[c * NSH + 48 * WSLOT : (c + 1) * NSH]
        return out

    sq_pad = pad_npad(np.sqrt(deg_new).astype(np.float32))       # [8, NPAD]
    d1_pad = pad_npad((dinv_new ** 2).astype(np.float64))
    d2_pad = pad_npad(dinv_new.astype(np.float64))
    top_pad = pad_npad(top_new.astype(np.float64))               # [8,NPAD,4]
    bat_pad = pad_npad(batch_new)
    # mark pad slots: zero scales, selg zero
    padmask = pad_npad(np.ones(N, np.float64))

    d1_pad *= padmask
    d2_pad *= padmask

    # host gate scales: softmax((top @ Wg.T)/TEMP) * dinv^p, [8, NPAD, EXP]
    def gate_scales(Wg, dpow):
        logit = top_pad @ Wg.T.astype(np.float64)                # [8,NPAD,EXP]
        e = np.exp(logit / TEMP)
        sm = e / e.sum(axis=-1, keepdims=True)
        return (sm * dpow[:, :, None]).astype(np.float32)

    sc0 = gate_scales(Wg0, d1_pad)
    sc1 = gate_scales(Wg1, d2_pad)

    wall0 = W0.transpose(1, 0, 2).reshape(F, EXP * HID).copy()
    wall1 = W1.transpose(1, 0, 2).reshape(F, EXP * HID).copy()
    ball0 = b0.reshape(1, EXP * HID).copy()
    ball1 = b1.reshape(1, EXP * HID).copy()

    in_maps = []
    for c in range(W_CORES):
        selg_c = np.zeros((WPC, 128, G), np.float32)
        bm = bat_pad[c].reshape(WPC, 128)
        pm = padmask[c].reshape(WPC, 128)
        wv, pv = np.nonzero(pm > 0)
        selg_c[wv, pv, bm[wv, pv]] = 1.0
        ts = plan["tok_src"][c]
        tok0_c = np.zeros((128, ts.shape[0], F), ml_dtypes.bfloat16)
        chv, pv2 = np.nonzero(ts >= 0)
        tok0_c[pv2, chv] = xs[ts[chv, pv2]]
        # scales / selg in partition-major [128, WPC, *] layout
        sc0_c = sc0[c].reshape(WPC, 128, EXP).transpose(1, 0, 2).reshape(128, WPC * EXP)
        sc1_c = sc1[c].reshape(WPC, 128, EXP).transpose(1, 0, 2).reshape(128, WPC * EXP)
        selg_pm = selg_c.transpose(1, 0, 2).reshape(128, WPC * G)
        in_maps.append({
            "tok0": tok0_c,
            "idxs": plan["idx"][c],
            "sels": plan["sel"][c].reshape(128, totch * 128),
            "wall0": wall0.astype(ml_dtypes.bfloat16),
            "wall1": wall1.astype(ml_dtypes.bfloat16),
            "ball0": ball0.astype(ml_dtypes.bfloat16),
            "ball1": ball1.astype(ml_dtypes.bfloat16),
            "sqdeg": sq_pad[c][None, :].astype(ml_dtypes.bfloat16),
            "scal0": np.ascontiguousarray(sc0_c),
            "scal1": np.ascontiguousarray(sc1_c),
            "selg": np.ascontiguousarray(selg_pm).astype(ml_dtypes.bfloat16),
            "wf": Wf.copy(),
        })

    from concourse.bass_utils import run_bass_kernel_spmd

    nc = _build_nc(RA, RB, totch)
    trace = os.environ.get("KERNEL_TRACE", "0") == "1"
    ncores = int(os.environ.get("KERNEL_CORES", str(W_CORES)))
    res = run_bass_kernel_spmd(nc, in_maps[:ncores], core_ids=list(range(ncores)),
                               trace=trace)
    kernel.last_results = res

    total = np.zeros((G, OUT), np.float64)
    for c in range(W_CORES):
        total += res.results[c]["pout"].astype(np.float64)
    cnt = np.maximum(plan["cnt"], 1.0)
    out = total / cnt[:, None] + bf.astype(np.float64)[None, :]
    return out.astype(np.float32)
